# revision 1
# baseline (speedup 1.0000x reference)
"""Trainium2 Bass kernel for the DH-LIF node single-step forward.

Math: the mask is one-hot over the branch dim NB, so

    spike = ( (1-beta) * (x @ (W + 0.5*M_eff).T + b) >= 1 )
    M_eff[h,i] = oma[h, idx[h,i]],   oma[h,k] = 0.5*(1 - sigmoid(tau_n[h,k]))

where idx[h,i] is the branch the (dense, one-hot) mask assigns to input i of
hidden unit h.  The host losslessly re-encodes the one-hot mask as that index
plane (fp8, values 0..3); the device reconstructs M_eff with a per-partition
cubic through the 4 oma values (exact at the integer nodes), builds
Wc = W + M_eff, transposes it to i-major on TensorE, splits hi/lo bf16 (keeps
fp32-level precision at bf16 matmul speed), and accumulates
out[h, b] = Wc_T.T @ x_T over 32 k-chunks.  Threshold compares against the
per-partition value 1/(1-beta) - b.

Sharding: hidden dim split across 8 cores (h_loc = 256); x replicated.
Host does layout/dtype prep (transpose of x, index extraction, sharding) and
the final gather/transpose.
"""

import numpy as np
import ml_dtypes

B, I, H, NB = 512, 4096, 2048, 4
NCORES = 8
H_LOC = H // NCORES          # 256
N_HT = H_LOC // 128          # 2 partition tiles of hidden per core
S = 512                      # i-supertile size for the Wc build
N_SUPER = I // S             # supers per h-tile
G = 512                      # transpose/psum group width (4x 128x128 tiles)
N_GROUPS = S // G            # groups per supertile
N_CHUNK = I // 128           # 32 matmul k-chunks

TRACE = False
LAST_RESULTS = None
_CACHED = {}


def _build_bass(reps=1):
    import concourse.bacc as bacc
    import concourse.mybir as mybir
    from concourse.tile import TileContext
    from concourse.masks import make_identity

    f32 = mybir.dt.float32
    bf16 = mybir.dt.bfloat16
    fp8 = mybir.dt.float8e4
    AF = mybir.ActivationFunctionType
    ALU = mybir.AluOpType

    nc = bacc.Bacc("TRN2", target_bir_lowering=False, debug=False)

    xT = nc.dram_tensor("xT", [I, B], bf16, kind="ExternalInput")
    w_in = nc.dram_tensor("w", [H_LOC, I], f32, kind="ExternalInput")
    idx_in = nc.dram_tensor("idx", [H_LOC, I], fp8, kind="ExternalInput")
    tau_n = nc.dram_tensor("tau_n", [H_LOC, NB], f32, kind="ExternalInput")
    tau_m = nc.dram_tensor("tau_m", [H_LOC, 1], f32, kind="ExternalInput")
    b_in = nc.dram_tensor("b", [H_LOC, 1], f32, kind="ExternalInput")
    out = nc.dram_tensor("out", [H_LOC, B], f32, kind="ExternalOutput")

    # x viewed as [chunk-groups, 128, 4, 512] for SBUF tiles
    xT_v = xT.rearrange("(g j p) b -> g p j b", p=128, j=4)
    n_xg = xT_v.shape[0]  # 8

    with TileContext(nc) as tc:
        with (
            tc.tile_pool(name="const", bufs=1) as const_pool,
            tc.tile_pool(name="xp", bufs=n_xg) as x_pool,
            tc.tile_pool(name="ix", bufs=N_HT * N_SUPER) as idx_pool,
            tc.tile_pool(name="wp", bufs=N_HT * N_SUPER) as w_pool,
            tc.tile_pool(name="hb", bufs=4) as h_pool,
            tc.tile_pool(name="ub", bufs=4) as u_pool,
            tc.tile_pool(name="hi", bufs=4) as hi_pool,
            tc.tile_pool(name="lo", bufs=4) as lo_pool,
            tc.tile_pool(name="res", bufs=2) as res_pool,
            tc.tile_pool(name="pt", bufs=3, space="PSUM") as psum_t_pool,
            tc.tile_pool(name="po", bufs=2, space="PSUM") as psum_o_pool,
            tc.tile_pool(name="pw", bufs=1, space="PSUM") as psum_w_pool,
        ):
            ident = const_pool.tile([128, 128], f32)
            make_identity(nc, ident)

            # HAM warmup: the PE sits idle for the first ~10us while Wc is
            # built, and its clock gate (PE_HAM) would hold it at 1.2 GHz for
            # the first ~3.4us of real matmuls.  Fill the idle window with
            # dummy matmuls so the array enters the kernel warm (2.4 GHz).
            warm = psum_w_pool.tile([128, 128], f32, name="warm")
            for wi in range(20):
                nc.tensor.matmul(warm[:], ident[:], ident[:],
                                 start=True, stop=True, skip_group_check=True)
            for rep in range(reps):
                _emit_rep(nc, tc, rep, ident,
                          const_pool, x_pool, idx_pool, w_pool, h_pool,
                          u_pool, hi_pool, lo_pool, res_pool,
                          psum_t_pool, psum_o_pool,
                          xT_v, n_xg, w_in, idx_in, tau_n, tau_m, b_in, out,
                          f32, bf16, fp8, AF, ALU)

    nc.compile()
    return nc


def _emit_rep(nc, tc, rep, ident,
              const_pool, x_pool, idx_pool, w_pool, h_pool,
              u_pool, hi_pool, lo_pool, res_pool,
              psum_t_pool, psum_o_pool,
              xT_v, n_xg, w_in, idx_in, tau_n, tau_m, b_in, out,
              f32, bf16, fp8, AF, ALU):
    R = f"r{rep}_"

    # tiny parameter DMAs first so they land ahead of the bulk traffic in
    # the DMA queues — the whole Wc build depends on them
    param_tiles = []
    for ht in range(N_HT):
        hs = slice(ht * 128, (ht + 1) * 128)
        tn = const_pool.tile([128, NB], f32, tag=f"{R}tn{ht}", name=f"{R}tn{ht}")
        tm = const_pool.tile([128, 1], f32, tag=f"{R}tm{ht}", name=f"{R}tm{ht}")
        bv = const_pool.tile([128, 1], f32, tag=f"{R}bv{ht}", name=f"{R}bv{ht}")
        nc.sync.dma_start(tn[:], tau_n[hs, :])
        nc.sync.dma_start(tm[:], tau_m[hs, :])
        nc.sync.dma_start(bv[:], b_in[hs, :])
        param_tiles.append((tn, tm, bv))

    # Pre-allocate all idx/W tiles and emit their DMAs interleaved with the
    # x tiles, first supertile first, so the Wc build can start immediately
    # while x streams in behind it.
    x_sb = [None] * n_xg
    idx_sb = {}
    w_sb = {}
    xg_next = [0]

    def dma_x(n):
        for _ in range(n):
            if xg_next[0] < n_xg:
                g = xg_next[0]
                xt = x_pool.tile([128, 4, B], bf16, tag="xsb", name=f"{R}x{g}")
                nc.sync.dma_start(xt[:], xT_v[g])
                x_sb[g] = xt
                xg_next[0] += 1

    for ht in range(N_HT):
        hs = slice(ht * 128, (ht + 1) * 128)
        for ig in range(N_SUPER):
            isl = slice(ig * S, (ig + 1) * S)
            it = idx_pool.tile([128, S], fp8, tag="ix", name=f"{R}ix{ht}_{ig}")
            nc.sync.dma_start(it[:], idx_in[hs, isl])
            wt = w_pool.tile([128, S], f32, tag="wp", name=f"{R}w{ht}_{ig}")
            nc.sync.dma_start(wt[:], w_in[hs, isl])
            idx_sb[(ht, ig)] = it
            w_sb[(ht, ig)] = wt
            dma_x(1)
    dma_x(n_xg)

    # per-h-tile parameters: polynomial coefs for M_eff and threshold
    coef = []   # (a_ap, b_ap, c_ap, d_ap) per ht
    thr_t = []
    for ht in range(N_HT):
        tn, tm, bv = param_tiles[ht]
        sig_n = const_pool.tile([128, NB], f32, tag=f"{R}sn{ht}", name=f"{R}sn{ht}")
        nc.scalar.activation(sig_n[:], tn[:], AF.Sigmoid)
        oma = const_pool.tile([128, NB], f32, tag=f"{R}oma{ht}", name=f"{R}oma{ht}")
        # 0.5 * (1 - sigmoid(tau_n)) — includes the 0.5 dendritic scale
        nc.vector.tensor_scalar(oma[:], sig_n[:], -0.5, 0.5, op0=ALU.mult, op1=ALU.add)

        # Newton -> monomial coefficients of the cubic through (k, oma[:,k]),
        # k=0..3:  p(x) = a + b x + c x^2 + d x^3
        sc = const_pool.tile([128, 12], f32, tag=f"{R}sc{ht}", name=f"{R}sc{ht}")
        o = lambda k: oma[:, k : k + 1]
        d3 = sc[:, 0:3]                       # first differences
        dd2 = sc[:, 3:5]                      # second differences
        ddd = sc[:, 5:6]                      # third difference
        b_c, c_c, d_c = sc[:, 6:7], sc[:, 7:8], sc[:, 8:9]
        t1, t2 = sc[:, 9:10], sc[:, 10:11]
        nc.vector.tensor_tensor(d3, oma[:, 1:4], oma[:, 0:3], ALU.subtract)
        nc.vector.tensor_tensor(dd2, d3[:, 1:3], d3[:, 0:2], ALU.subtract)
        nc.vector.tensor_tensor(ddd, dd2[:, 1:2], dd2[:, 0:1], ALU.subtract)
        d0, dd0 = d3[:, 0:1], dd2[:, 0:1]
        # d = ddd/6, c = (dd0 - ddd)/2 first (they gate the Horner start)
        nc.scalar.mul(d_c, ddd, 1.0 / 6.0)
        nc.vector.tensor_tensor(c_c, dd0, ddd, ALU.subtract)
        nc.vector.tensor_scalar(c_c, c_c, 0.5, None, op0=ALU.mult)
        # b = d0 - dd0/2 + ddd/3
        nc.scalar.mul(t2, ddd, 1.0 / 3.0)
        nc.vector.tensor_scalar(t1, dd0, -0.5, None, op0=ALU.mult)
        nc.vector.tensor_tensor(t1, t1, d0, ALU.add)
        nc.vector.tensor_tensor(b_c, t1, t2, ALU.add)
        coef.append((o(0), b_c, c_c, d_c))

        beta = const_pool.tile([128, 1], f32, tag=f"{R}be{ht}", name=f"{R}be{ht}")
        nc.scalar.activation(beta[:], tm[:], AF.Sigmoid)
        omb = const_pool.tile([128, 1], f32, tag=f"{R}ob{ht}", name=f"{R}ob{ht}")
        nc.vector.tensor_scalar(omb[:], beta[:], -1.0, 1.0, op0=ALU.mult, op1=ALU.add)
        rb = const_pool.tile([128, 1], f32, tag=f"{R}rb{ht}", name=f"{R}rb{ht}")
        nc.vector.reciprocal(rb[:], omb[:])
        thr = const_pool.tile([128, 1], f32, tag=f"{R}th{ht}", name=f"{R}th{ht}")
        nc.vector.tensor_tensor(thr[:], rb[:], bv[:], ALU.subtract)
        thr_t.append(thr)

    # main pipeline
    SKEW = 2
    pending = []  # [(hi_tile, lo_tile, base_chunk, ht)]
    psum_out = [None] * N_HT

    def issue_mms(pend):
        hi_t, lo_t, base_c, ht_ = pend
        po = psum_out[ht_]
        is_last_group = base_c == (N_CHUNK - 4)
        for j in range(4):
            c = base_c + j
            xg, xj = divmod(c, 4)
            nc.tensor.matmul(
                po[:], hi_t[:, j * 128 : (j + 1) * 128],
                x_sb[xg][:, xj, :],
                start=(c == 0), stop=False, skip_group_check=True,
            )
            nc.tensor.matmul(
                po[:], lo_t[:, j * 128 : (j + 1) * 128],
                x_sb[xg][:, xj, :],
                start=False, stop=(is_last_group and j == 3),
                skip_group_check=True,
            )

    # Software-pipelined emission over all (ht, ig) supers, in the same order
    # as their DMAs were issued.
    supers = [(ht, ig) for ht in range(N_HT) for ig in range(N_SUPER)]
    n_sup = len(supers)
    P_t = [None] * n_sup

    for ht in range(N_HT):
        psum_out[ht] = psum_o_pool.tile([128, B], f32, tag="po", name=f"{R}po{ht}")

    def s0_ts1(k):  # DVE: P = d*idx + c  (2x mode; Pool keeps only TT4)
        ht, ig = supers[k]
        _, _, c_ap, d_ap = coef[ht]
        P = h_pool.tile([128, S], f32, tag="hb", name=f"{R}P{k}")
        P_t[k] = P
        nc.vector.tensor_scalar(P[:], idx_sb[(ht, ig)][:], d_ap, c_ap,
                                op0=ALU.mult, op1=ALU.add)

    def s1_tt1(k):  # DVE: P *= idx
        ht, ig = supers[k]
        nc.vector.tensor_tensor(P_t[k][:], P_t[k][:], idx_sb[(ht, ig)][:], ALU.mult)

    def s2_addb(k):  # Act: P += b
        ht, ig = supers[k]
        nc.scalar.add(P_t[k][:], P_t[k][:], coef[ht][1])

    def s3_tt2(k):  # DVE: P *= idx
        ht, ig = supers[k]
        nc.vector.tensor_tensor(P_t[k][:], P_t[k][:], idx_sb[(ht, ig)][:], ALU.mult)

    def s4_addw(k):  # Pool: P += W
        ht, ig = supers[k]
        nc.gpsimd.tensor_tensor(P_t[k][:], P_t[k][:], w_sb[(ht, ig)][:], ALU.add)

    def s5_consume(k):  # Act +a into wc + PE transposes + hi/lo + MMs
        ht, ig = supers[k]
        a_ap = coef[ht][0]
        wctile = u_pool.tile([128, S], f32, tag="ub", name=f"{R}wc{k}")
        for g in range(N_GROUPS):
            gsl = slice(g * G, (g + 1) * G)
            nc.scalar.add(wctile[:, gsl], P_t[k][:, gsl], a_ap)
            pt = psum_t_pool.tile([128, G], f32, tag="pt", name=f"{R}pt{k}_{g}")
            for j in range(4):
                col = g * G + j * 128
                nc.tensor.transpose(
                    pt[:, j * 128 : (j + 1) * 128],
                    wctile[:, col : col + 128],
                    ident[:],
                )
            hi_t = hi_pool.tile([128, G], bf16, tag="hi", name=f"{R}hi{k}_{g}")
            lo_t = lo_pool.tile([128, G], bf16, tag="lo", name=f"{R}lo{k}_{g}")
            nc.scalar.copy(hi_t[:], pt[:])
            nc.vector.tensor_tensor(lo_t[:], pt[:], hi_t[:], ALU.subtract)
            pending.append((hi_t, lo_t, (ig * N_GROUPS + g) * 4, ht))
            if len(pending) > SKEW:
                issue_mms(pending.pop(0))

    # prologue
    s0_ts1(0)
    s0_ts1(1)
    s1_tt1(0)
    for k in range(n_sup):
        s2_addb(k)
        if k + 1 < n_sup:
            s1_tt1(k + 1)
        s3_tt2(k)
        if k + 2 < n_sup:
            s0_ts1(k + 2)
        s4_addw(k)
        s5_consume(k)

    for pend in pending:
        issue_mms(pend)

    for ht in range(N_HT):
        res = res_pool.tile([128, B], f32, tag="res", name=f"{R}res{ht}")
        nc.vector.tensor_scalar(
            res[:], psum_out[ht][:], thr_t[ht][:], None, op0=ALU.is_ge
        )
        nc.sync.dma_start(out[ht * 128 : (ht + 1) * 128, :], res[:])


def _get_nc(reps=1):
    key = f"nc{reps}"
    if key not in _CACHED:
        _CACHED[key] = _build_bass(reps)
    return _CACHED[key]


def kernel(**inputs):
    global LAST_RESULTS
    from concourse.bass_utils import run_bass_kernel_spmd

    x = np.asarray(inputs["x"], dtype=np.float32)
    W = np.asarray(inputs["W"], dtype=np.float32)
    b = np.asarray(inputs["b"], dtype=np.float32)
    tau_m = np.asarray(inputs["tau_m"], dtype=np.float32)
    tau_n = np.asarray(inputs["tau_n"], dtype=np.float32)
    mask = np.asarray(inputs["mask"], dtype=np.float32)

    bf16 = ml_dtypes.bfloat16
    fp8 = ml_dtypes.float8_e4m3
    xT = np.ascontiguousarray(x.T).astype(bf16)                      # [I, B]
    # branch index of each (h, i): mask is one-hot over k (exact 0/1 values)
    idx = (
        mask[:, :, 1] + 2.0 * mask[:, :, 2] + 3.0 * mask[:, :, 3]
    ).astype(fp8)                                                     # [H, I]

    nc = _get_nc()
    in_maps = []
    for c in range(NCORES):
        hs = slice(c * H_LOC, (c + 1) * H_LOC)
        in_maps.append({
            "xT": xT,
            "w": np.ascontiguousarray(W[hs]),
            "idx": np.ascontiguousarray(idx[hs]),
            "tau_n": np.ascontiguousarray(tau_n[hs]),
            "tau_m": np.ascontiguousarray(tau_m[hs, None]),
            "b": np.ascontiguousarray(b[hs, None]),
        })

    try:
        res = run_bass_kernel_spmd(
            nc, in_maps, core_ids=list(range(NCORES)), trace=TRACE,
        )
    except Exception:
        if not TRACE:
            raise
        # tracing needs the NTFF profiling hook, which not every
        # environment provides — rerun without it
        res = run_bass_kernel_spmd(
            nc, in_maps, core_ids=list(range(NCORES)), trace=False,
        )
    LAST_RESULTS = res
    outT = np.concatenate([r["out"] for r in res.results], axis=0)   # [H, B]
    return np.ascontiguousarray(outT.T)                               # [B, H]



# revision 5
# speedup vs baseline: 1.1915x; 1.1915x over previous
"""Trainium2 Bass kernel for the DH-LIF node single-step forward.

Math: the mask is one-hot over the branch dim NB, so

    spike = ( x @ (W + M).T + b >= 1/(1-beta) ),
    M[h,i] = oma[h, idx[h,i]],  oma[h,k] = 0.5*(1 - sigmoid(tau_n[h,k]))

The 2-bit branch index idx is re-encoded on the host (losslessly) as two
{0,1} bit planes t1 = idx&1, t2 = idx>>1, shipped transposed (i-major) in
fp8.  On device, M is reconstructed in the bilinear-bit basis

    M = A + B*t1 + C*t2 + D*t1*t2        (A..D per-h, from sigmoid(tau_n))

evaluated entirely in fp16 on the DVE (4x mode): multiplies by {0,1} are
exact, so precision matches a single-rounding f32 build.  Everything is
scaled by 256 to keep fp16 away from subnormals.  W ships pre-transposed,
pre-scaled fp16; x ships transposed fp8 (spikes are exact in fp8).  One
fp16[lhsT] x fp8[rhs] matmul pass accumulates out[h,b] over 32 k-chunks.
Threshold compares PSUM against 256/(1-beta) - 256*b per partition.

Per-h coefficient rows (A..D) are broadcast across partitions with tiny
rank-1 PE matmuls (ones[1,128].T @ row[1,256]), then used by the big DVE
ops through a stride-0 repeat access pattern.

Sharding: hidden dim split across 8 cores (h_loc = 256); x replicated.
Host does layout/dtype prep (bit-plane extraction, transposes, packing)
and the final gather/cast.
"""

import numpy as np
import ml_dtypes

B, I, H, NB = 512, 4096, 2048, 4
NCORES = 8
H_LOC = H // NCORES          # 256
N_HT = H_LOC // 128          # 2 h-tiles of 128
N_CHUNK = I // 128           # 32 matmul k-chunks
CPQ = 8                      # k-chunks per DMA/compute super-chunk
NQ = N_CHUNK // CPQ          # 4 super-chunks
SCALE = 256.0                # fp16 subnormal guard (power of 2, lossless)

TRACE = False
LAST_RESULTS = None
_CACHED = {}


def _rep_ap(bass_mod, tile_ap, reps):
    """[128, 256] coeff tile -> [128, reps, 256] stride-0 repeat AP."""
    return bass_mod.AP(
        tile_ap.tensor, tile_ap.offset,
        [list(tile_ap.ap[0]), [0, reps], [1, 256]],
    )


def _build_bass():
    import concourse.bacc as bacc
    import concourse.bass as bass
    import concourse.mybir as mybir
    from concourse.tile import TileContext
    from concourse.masks import make_identity

    f32 = mybir.dt.float32
    f16 = mybir.dt.float16
    fp8 = mybir.dt.float8e4
    AF = mybir.ActivationFunctionType
    ALU = mybir.AluOpType

    nc = bacc.Bacc("TRN2", target_bir_lowering=False, debug=False)

    # host-packed layouts: col block c holds k-chunk c (i = c*128 + p)
    t1_d = nc.dram_tensor("t1", [128, N_CHUNK * H_LOC], fp8, kind="ExternalInput")
    t2_d = nc.dram_tensor("t2", [128, N_CHUNK * H_LOC], fp8, kind="ExternalInput")
    w_d = nc.dram_tensor("w", [128, N_CHUNK * H_LOC], f16, kind="ExternalInput")
    x_d = nc.dram_tensor("x", [128, N_CHUNK * B], fp8, kind="ExternalInput")
    par_d = nc.dram_tensor("par", [128, 6 * N_HT], f32, kind="ExternalInput")
    out_d = nc.dram_tensor("out", [H_LOC, B], fp8, kind="ExternalOutput")

    FQ = CPQ * H_LOC         # 2048 free elems per super-chunk (t/w planes)
    FXQ = CPQ * B            # 4096 free elems per super-chunk (x)

    with TileContext(nc) as tc:
        with (
            tc.tile_pool(name="const", bufs=1) as cpool,
            tc.tile_pool(name="strm", bufs=1) as spool,
            tc.tile_pool(name="po", bufs=2, space="PSUM") as psum_o,
            tc.tile_pool(name="pt", bufs=2, space="PSUM") as psum_t,
            tc.tile_pool(name="pw", bufs=1, space="PSUM") as psum_w,
        ):
            # ---- bulk DMAs, super-chunk interleaved, issue first ----
            par = cpool.tile([128, 6 * N_HT], f32)
            nc.sync.dma_start(par[:], par_d[:, :])
            t1_8, t2_8, w16, x8 = [], [], [], []
            for q in range(NQ):
                sl = slice(q * FQ, (q + 1) * FQ)
                xsl = slice(q * FXQ, (q + 1) * FXQ)
                a = spool.tile([128, FQ], fp8, tag=f"t1_{q}")
                bq = spool.tile([128, FQ], fp8, tag=f"t2_{q}")
                wq = spool.tile([128, FQ], f16, tag=f"w_{q}")
                xq = spool.tile([128, FXQ], fp8, tag=f"x_{q}")
                nc.sync.dma_start(a[:], t1_d[:, sl])
                nc.sync.dma_start(bq[:], t2_d[:, sl])
                nc.sync.dma_start(wq[:], w_d[:, sl])
                nc.sync.dma_start(xq[:], x_d[:, xsl])
                t1_8.append(a); t2_8.append(bq); w16.append(wq); x8.append(xq)

            # ---- PE warmup: keep HAM at full clock until real matmuls ----
            ident = cpool.tile([128, 128], f32)
            make_identity(nc, ident)
            warm = psum_w.tile([128, 128], f32, name="warm")
            for _ in range(20):
                nc.tensor.matmul(warm[:], ident[:], ident[:],
                                 start=True, stop=True, skip_group_check=True)

            # ---- per-h params -> bilinear coeffs (scaled), thresholds ----
            cf = []     # [128, 4] f32 coeff tile per ht (cols A,B,C,D)
            thr = []    # [128, 1] f32 per ht
            for ht in range(N_HT):
                p0 = 6 * ht
                sig = cpool.tile([128, 4], f32, tag=f"sig{ht}")
                nc.scalar.activation(sig[:], par[:, p0:p0 + 4], AF.Sigmoid)
                oma = cpool.tile([128, 4], f32, tag=f"oma{ht}")
                # SCALE * 0.5 * (1 - sig)
                nc.vector.tensor_scalar(oma[:], sig[:], -0.5 * SCALE, 0.5 * SCALE,
                                        op0=ALU.mult, op1=ALU.add)
                c = cpool.tile([128, 4], f32, tag=f"cf{ht}")
                o = lambda k: oma[:, k:k + 1]
                nc.vector.tensor_copy(c[:, 0:1], o(0))                       # A
                nc.vector.tensor_tensor(c[:, 1:2], o(1), o(0), ALU.subtract)  # B
                nc.vector.tensor_tensor(c[:, 2:3], o(2), o(0), ALU.subtract)  # C
                t = cpool.tile([128, 1], f32, tag=f"cft{ht}")
                nc.vector.tensor_tensor(t[:], o(3), o(2), ALU.subtract)
                nc.vector.tensor_tensor(c[:, 3:4], t[:], c[:, 1:2], ALU.subtract)  # D
                cf.append(c)

                sigm = cpool.tile([128, 1], f32, tag=f"sm{ht}")
                nc.scalar.activation(sigm[:], par[:, p0 + 4:p0 + 5], AF.Sigmoid)
                omb = cpool.tile([128, 1], f32, tag=f"ob{ht}")
                nc.vector.tensor_scalar(omb[:], sigm[:], -1.0, 1.0,
                                        op0=ALU.mult, op1=ALU.add)
                rcp = cpool.tile([128, 1], f32, tag=f"rc{ht}")
                nc.vector.reciprocal(rcp[:], omb[:])
                # thr = SCALE/(1-beta) - SCALE*b
                tb = cpool.tile([128, 1], f32, tag=f"tb{ht}")
                nc.vector.tensor_scalar(tb[:], par[:, p0 + 5:p0 + 6], SCALE, None,
                                        op0=ALU.mult)
                th = cpool.tile([128, 1], f32, tag=f"th{ht}")
                nc.vector.scalar_tensor_tensor(th[:], rcp[:], SCALE, tb[:],
                                               ALU.mult, ALU.subtract)
                thr.append(th)

            # ---- replicate coeff columns to [128, 256] fp16 row tiles ----
            ones = cpool.tile([1, 128], f32)
            nc.vector.memset(ones[:], 1.0)
            rep = []
            for k in range(4):
                pr = psum_t.tile([1, H_LOC], f32, tag="pr", name=f"pr{k}")
                for ht in range(N_HT):
                    nc.tensor.transpose(pr[:, ht * 128:(ht + 1) * 128],
                                        cf[ht][:, k:k + 1], ident[:])
                row = cpool.tile([1, H_LOC], f32, tag=f"row{k}")
                nc.vector.tensor_copy(row[:], pr[:])
                pb = psum_t.tile([128, H_LOC], f32, tag="pb", name=f"pb{k}")
                nc.tensor.matmul(pb[:], ones[:], row[:],
                                 start=True, stop=True, skip_group_check=True)
                r = cpool.tile([128, H_LOC], f16, tag=f"rep{k}")
                nc.vector.tensor_copy(r[:], pb[:])
                rep.append(r)
            repA = _rep_ap(bass, rep[0][:], CPQ)
            repB = _rep_ap(bass, rep[1][:], CPQ)
            repC = _rep_ap(bass, rep[2][:], CPQ)
            repD = _rep_ap(bass, rep[3][:], CPQ)

            po = [psum_o.tile([128, B], f32, tag="po", name=f"po{ht}")
                  for ht in range(N_HT)]

            # ---- streamed build + matmuls ----
            for q in range(NQ):
                t1q = spool.tile([128, FQ], f16, tag=f"t1f_{q}")
                nc.scalar.copy(t1q[:], t1_8[q][:])                    # Act cvt
                t2q = spool.tile([128, FQ], f16, tag=f"t2f_{q}")
                nc.gpsimd.tensor_copy(t2q[:], t2_8[q][:])             # Pool cvt
                v = lambda ap: ap.rearrange("p (c h) -> p c h", c=CPQ)

                wa = spool.tile([128, FQ], f16, tag=f"wa_{q}")
                nc.vector.tensor_tensor(v(wa[:]), v(w16[q][:]), repA, ALU.add)
                P = spool.tile([128, FQ], f16, tag=f"P_{q}")
                nc.vector.tensor_tensor(v(P[:]), v(t1q[:]), repB, ALU.mult)
                nc.vector.tensor_tensor(P[:], P[:], wa[:], ALU.add)
                Q = spool.tile([128, FQ], f16, tag=f"Q_{q}")
                nc.vector.tensor_tensor(v(Q[:]), v(t1q[:]), repD, ALU.mult)
                nc.vector.tensor_tensor(v(Q[:]), v(Q[:]), repC, ALU.add)
                nc.vector.tensor_tensor(Q[:], Q[:], t2q[:], ALU.mult)
                nc.vector.tensor_tensor(P[:], P[:], Q[:], ALU.add)

                for c in range(CPQ):
                    gc = q * CPQ + c
                    for ht in range(N_HT):
                        nc.tensor.matmul(
                            po[ht][:],
                            P[:, c * H_LOC + ht * 128: c * H_LOC + (ht + 1) * 128],
                            x8[q][:, c * B:(c + 1) * B],
                            start=(gc == 0), stop=(gc == N_CHUNK - 1),
                            skip_group_check=True,
                        )

            # ---- threshold + store ----
            for ht in range(N_HT):
                res = cpool.tile([128, B], fp8, tag=f"res{ht}")
                nc.vector.tensor_scalar(res[:], po[ht][:], thr[ht][:], None,
                                        op0=ALU.is_ge)
                nc.sync.dma_start(out_d[ht * 128:(ht + 1) * 128, :], res[:])

    nc.compile()
    return nc


def _get_nc(reps=1):
    key = "nc"
    if key not in _CACHED:
        _CACHED[key] = _build_bass()
    return _CACHED[key]


def _pack_chunks(arr_T, width):
    """[I, width] i-major -> [128, N_CHUNK*width] with col block c = chunk c."""
    return np.ascontiguousarray(
        arr_T.reshape(N_CHUNK, 128, width).transpose(1, 0, 2).reshape(128, N_CHUNK * width)
    )


def kernel(**inputs):
    global LAST_RESULTS
    from concourse.bass_utils import run_bass_kernel_spmd

    x = np.asarray(inputs["x"], dtype=np.float32)
    W = np.asarray(inputs["W"], dtype=np.float32)
    b = np.asarray(inputs["b"], dtype=np.float32)
    tau_m = np.asarray(inputs["tau_m"], dtype=np.float32)
    tau_n = np.asarray(inputs["tau_n"], dtype=np.float32)
    mask = np.asarray(inputs["mask"], dtype=np.float32)

    fp8 = ml_dtypes.float8_e4m3
    idx = (mask[:, :, 1] + 2.0 * mask[:, :, 2] + 3.0 * mask[:, :, 3]).astype(np.int8)
    t1 = (idx & 1).astype(np.float32)      # [H, I]
    t2 = (idx >> 1).astype(np.float32)
    xp = _pack_chunks(np.ascontiguousarray(x.T), B).astype(fp8)   # [128, 32*512]

    nc = _get_nc()
    in_maps = []
    for c in range(NCORES):
        hs = slice(c * H_LOC, (c + 1) * H_LOC)
        par = np.zeros((128, 6 * N_HT), dtype=np.float32)
        for ht in range(N_HT):
            hh = slice(c * H_LOC + ht * 128, c * H_LOC + (ht + 1) * 128)
            par[:, 6 * ht:6 * ht + 4] = tau_n[hh]
            par[:, 6 * ht + 4] = tau_m[hh]
            par[:, 6 * ht + 5] = b[hh]
        in_maps.append({
            "t1": _pack_chunks(np.ascontiguousarray(t1[hs].T), H_LOC).astype(fp8),
            "t2": _pack_chunks(np.ascontiguousarray(t2[hs].T), H_LOC).astype(fp8),
            "w": _pack_chunks(np.ascontiguousarray(W[hs].T) * np.float32(SCALE),
                              H_LOC).astype(np.float16),
            "x": xp,
            "par": par,
        })

    try:
        res = run_bass_kernel_spmd(
            nc, in_maps, core_ids=list(range(NCORES)), trace=TRACE,
        )
    except Exception:
        if not TRACE:
            raise
        res = run_bass_kernel_spmd(
            nc, in_maps, core_ids=list(range(NCORES)), trace=False,
        )
    LAST_RESULTS = res
    outT = np.concatenate([r["out"].astype(np.float32) for r in res.results], axis=0)
    return np.ascontiguousarray(outT.T)                               # [B, H]


# revision 8
# speedup vs baseline: 1.8188x; 1.5264x over previous
"""Trainium2 Bass kernel for the DH-LIF node single-step forward.

Math: the mask is one-hot over the branch dim NB, so

    spike = ( x @ (W + M).T + b >= 1/(1-beta) ),
    M[h,i] = oma[h, idx[h,i]],  oma[h,k] = 0.5*(1 - sigmoid(tau_n[h,k]))

The 2-bit branch index idx is re-encoded on the host (losslessly) as two
{0,1} bit planes t1 = idx&1, t2 = idx>>1 (fp8, h-major).  On device, M is
built per h-tile in the bilinear-bit basis with per-partition coefficients

    M = (A + B*t1) + (C + D*t1)*t2        (A..D per-h from sigmoid(tau_n))

as two DVE tensor_scalar ops (4x mode, dual per-partition scalars) and two
tensor_tensor ops (2x); multiplies by {0,1} are exact in fp16, so precision
matches a single-rounding f32 build.  Everything is scaled by 256 to keep
fp16 away from subnormals.  The [h,i] result is flipped to i-major with PE
transposes; the PSUM->SBUF readback is fused with the +W pass (gpsimd
tensor_tensor against host-pre-transposed fp16 W), giving the matmul lhsT.
x ships transposed fp8 (spikes exact in fp8); one fp16 x fp8 matmul pass
accumulates out[h,b] over 32 k-chunks; PSUM is thresholded against
256/(1-beta) - 256*b per partition.  Dummy PE matmuls pad pipeline gaps so
the tensor engine's activity-gated clock stays at full rate.

Sharding: hidden dim split across 8 cores (h_loc = 256); x replicated.
Host does layout/dtype prep (bit-plane extraction, transposes, packing)
and the final gather/cast.
"""

import numpy as np
import ml_dtypes

B, I, H, NB = 512, 4096, 2048, 4
NCORES = 8
H_LOC = H // NCORES          # 256
N_HT = H_LOC // 128          # 2 h-tiles of 128
N_CHUNK = I // 128           # 32 matmul k-chunks
NQ = 4                       # DMA/compute super-chunks
CPQ = N_CHUNK // NQ          # 8 k-chunks per super-chunk
IQ = CPQ * 128               # 1024 i per super-chunk
SCALE = 256.0                # fp16 subnormal guard (power of 2, lossless)
N_WARM = 20                  # initial PE warmup matmuls
N_FILL = 12                  # dummy matmuls per (q,ht) to keep PE clock hot

TRACE = False
LAST_RESULTS = None
_CACHED = {}


def _build_bass():
    import concourse.bacc as bacc
    import concourse.mybir as mybir
    from concourse.tile import TileContext
    from concourse.masks import make_identity

    f32 = mybir.dt.float32
    f16 = mybir.dt.float16
    fp8 = mybir.dt.float8e4
    AF = mybir.ActivationFunctionType
    ALU = mybir.AluOpType

    nc = bacc.Bacc("TRN2", target_bir_lowering=False, debug=False)

    # h-major bit planes: [128, (ht, i)]
    t1_d = nc.dram_tensor("t1", [128, N_HT * I], fp8, kind="ExternalInput")
    t2_d = nc.dram_tensor("t2", [128, N_HT * I], fp8, kind="ExternalInput")
    # i-major transposed W * SCALE: [128, (chunk, ht, h)]
    w_d = nc.dram_tensor("w", [128, N_CHUNK * H_LOC], f16, kind="ExternalInput")
    # i-major x: [128, (chunk, b)]
    x_d = nc.dram_tensor("x", [128, N_CHUNK * B], fp8, kind="ExternalInput")
    par_d = nc.dram_tensor("par", [128, 6 * N_HT], f32, kind="ExternalInput")
    out_d = nc.dram_tensor("out", [H_LOC, B], fp8, kind="ExternalOutput")

    t1_v = t1_d.rearrange("p (t i) -> p t i", t=N_HT)
    t2_v = t2_d.rearrange("p (t i) -> p t i", t=N_HT)

    with TileContext(nc) as tc:
        with (
            tc.tile_pool(name="const", bufs=1) as cpool,
            tc.tile_pool(name="strm", bufs=1) as spool,
            tc.tile_pool(name="po", bufs=2, space="PSUM") as psum_o,
            tc.tile_pool(name="pt", bufs=3, space="PSUM") as psum_t,
            tc.tile_pool(name="pw", bufs=1, space="PSUM") as psum_w,
        ):
            # ---- bulk DMAs, super-chunk interleaved, issue first ----
            par = cpool.tile([128, 6 * N_HT], f32)
            nc.sync.dma_start(par[:], par_d[:, :])
            t1_8, t2_8, w16, x8 = [], [], [], []
            for q in range(NQ):
                isl = slice(q * IQ, (q + 1) * IQ)
                wsl = slice(q * CPQ * H_LOC, (q + 1) * CPQ * H_LOC)
                xsl = slice(q * CPQ * B, (q + 1) * CPQ * B)
                a = spool.tile([128, N_HT, IQ], fp8, tag=f"t1_{q}")
                bq = spool.tile([128, N_HT, IQ], fp8, tag=f"t2_{q}")
                wq = spool.tile([128, CPQ * H_LOC], f16, tag=f"w_{q}")
                xq = spool.tile([128, CPQ * B], fp8, tag=f"x_{q}")
                nc.sync.dma_start(a[:], t1_v[:, :, isl])
                nc.sync.dma_start(bq[:], t2_v[:, :, isl])
                nc.sync.dma_start(wq[:], w_d[:, wsl])
                nc.sync.dma_start(xq[:], x_d[:, xsl])
                t1_8.append(a); t2_8.append(bq); w16.append(wq); x8.append(xq)

            # ---- PE warmup: keep HAM at full clock until real matmuls ----
            ident16 = cpool.tile([128, 128], f16)
            make_identity(nc, ident16)
            warm = psum_w.tile([128, 128], f32, name="warm")

            def fill_pe(n):
                for _ in range(n):
                    nc.tensor.matmul(warm[:], ident16[:], ident16[:],
                                     start=True, stop=True, skip_group_check=True)

            fill_pe(N_WARM)

            # ---- per-h params -> bilinear coeffs (scaled), thresholds ----
            coef = []   # (A, B, C, D) [128,1] f32 APs per ht
            thr = []
            for ht in range(N_HT):
                p0 = 6 * ht
                sig = cpool.tile([128, 4], f32, tag=f"sig{ht}")
                nc.scalar.activation(sig[:], par[:, p0:p0 + 4], AF.Sigmoid)
                oma = cpool.tile([128, 4], f32, tag=f"oma{ht}")
                # SCALE * 0.5 * (1 - sig)
                nc.vector.tensor_scalar(oma[:], sig[:], -0.5 * SCALE, 0.5 * SCALE,
                                        op0=ALU.mult, op1=ALU.add)
                c = cpool.tile([128, 4], f32, tag=f"cf{ht}")
                o = lambda k: oma[:, k:k + 1]
                nc.vector.tensor_copy(c[:, 0:1], o(0))                        # A
                nc.vector.tensor_tensor(c[:, 1:2], o(1), o(0), ALU.subtract)  # B
                nc.vector.tensor_tensor(c[:, 2:3], o(2), o(0), ALU.subtract)  # C
                t = cpool.tile([128, 1], f32, tag=f"cft{ht}")
                nc.vector.tensor_tensor(t[:], o(3), o(2), ALU.subtract)
                nc.vector.tensor_tensor(c[:, 3:4], t[:], c[:, 1:2], ALU.subtract)  # D
                coef.append((c[:, 0:1], c[:, 1:2], c[:, 2:3], c[:, 3:4]))

                sigm = cpool.tile([128, 1], f32, tag=f"sm{ht}")
                nc.scalar.activation(sigm[:], par[:, p0 + 4:p0 + 5], AF.Sigmoid)
                omb = cpool.tile([128, 1], f32, tag=f"ob{ht}")
                nc.vector.tensor_scalar(omb[:], sigm[:], -1.0, 1.0,
                                        op0=ALU.mult, op1=ALU.add)
                rcp = cpool.tile([128, 1], f32, tag=f"rc{ht}")
                nc.vector.reciprocal(rcp[:], omb[:])
                tb = cpool.tile([128, 1], f32, tag=f"tb{ht}")
                nc.vector.tensor_scalar(tb[:], par[:, p0 + 5:p0 + 6], SCALE, None,
                                        op0=ALU.mult)
                th = cpool.tile([128, 1], f32, tag=f"th{ht}")
                nc.vector.scalar_tensor_tensor(th[:], rcp[:], SCALE, tb[:],
                                               ALU.mult, ALU.subtract)
                thr.append(th)

            po = [psum_o.tile([128, B], f32, tag="po", name=f"po{ht}")
                  for ht in range(N_HT)]

            # ---- streamed build + transpose + matmuls ----
            for q in range(NQ):
                t1f = spool.tile([128, N_HT, IQ], f16, tag=f"t1f_{q}")
                nc.scalar.copy(t1f[:], t1_8[q][:])          # Act cvt (both ht)
                t2f = spool.tile([128, N_HT, IQ], f16, tag=f"t2f_{q}")
                nc.gpsimd.tensor_copy(t2f[:], t2_8[q][:])   # Pool cvt
                # w view: [p, c, ht, h]
                wv = w16[q][:].rearrange("p (c t h) -> p c t h", c=CPQ, t=N_HT)

                for ht in range(N_HT):
                    A, Bc, Cc, D = coef[ht]
                    t1h = t1f[:, ht, :]
                    Q = spool.tile([128, IQ], f16, tag=f"Q_{q}{ht}")
                    nc.vector.tensor_scalar(Q[:], t1h, D, Cc,
                                            op0=ALU.mult, op1=ALU.add)
                    nc.vector.tensor_tensor(Q[:], Q[:], t2f[:, ht, :], ALU.mult)
                    P = spool.tile([128, IQ], f16, tag=f"P_{q}{ht}")
                    nc.vector.tensor_scalar(P[:], t1h, Bc, A,
                                            op0=ALU.mult, op1=ALU.add)
                    nc.vector.tensor_tensor(P[:], P[:], Q[:], ALU.add)

                    pt = psum_t.tile([128, IQ], f16, tag="pt", name=f"pt{q}_{ht}")
                    for c in range(CPQ):
                        cs = slice(c * 128, (c + 1) * 128)
                        nc.tensor.transpose(pt[:, cs], P[:, cs], ident16[:])
                    # PSUM readback fused with +W (DVE; GPSIMD can't read PSUM)
                    wc = spool.tile([128, IQ], f16, tag=f"wc_{q}{ht}")
                    nc.vector.tensor_tensor(
                        wc[:].rearrange("p (c h) -> p c h", c=CPQ),
                        pt[:].rearrange("p (c h) -> p c h", c=CPQ),
                        wv[:, :, ht, :], ALU.add)

                    for c in range(CPQ):
                        gc = q * CPQ + c
                        nc.tensor.matmul(
                            po[ht][:],
                            wc[:, c * 128:(c + 1) * 128],
                            x8[q][:, c * B:(c + 1) * B],
                            start=(gc == 0), stop=(gc == N_CHUNK - 1),
                            skip_group_check=True,
                        )
                    fill_pe(N_FILL)

            # ---- threshold + store ----
            for ht in range(N_HT):
                res = cpool.tile([128, B], fp8, tag=f"res{ht}")
                nc.vector.tensor_scalar(res[:], po[ht][:], thr[ht][:], None,
                                        op0=ALU.is_ge)
                nc.sync.dma_start(out_d[ht * 128:(ht + 1) * 128, :], res[:])

    nc.compile()
    return nc


def _get_nc(reps=1):
    key = "nc"
    if key not in _CACHED:
        _CACHED[key] = _build_bass()
    return _CACHED[key]


def _pack_hmaj(arr):
    """[H_LOC, I] -> [128, N_HT*I] h-major (partition = h%128, ht blocks)."""
    return np.ascontiguousarray(
        arr.reshape(N_HT, 128, I).transpose(1, 0, 2).reshape(128, N_HT * I)
    )


def _pack_imaj(arr_T, width):
    """[I, width] i-major -> [128, N_CHUNK*width], col block c = k-chunk c."""
    return np.ascontiguousarray(
        arr_T.reshape(N_CHUNK, 128, width).transpose(1, 0, 2)
        .reshape(128, N_CHUNK * width)
    )


def kernel(**inputs):
    global LAST_RESULTS
    from concourse.bass_utils import run_bass_kernel_spmd

    x = np.asarray(inputs["x"], dtype=np.float32)
    W = np.asarray(inputs["W"], dtype=np.float32)
    b = np.asarray(inputs["b"], dtype=np.float32)
    tau_m = np.asarray(inputs["tau_m"], dtype=np.float32)
    tau_n = np.asarray(inputs["tau_n"], dtype=np.float32)
    mask = np.asarray(inputs["mask"], dtype=np.float32)

    fp8 = ml_dtypes.float8_e4m3
    idx = (mask[:, :, 1] + 2.0 * mask[:, :, 2] + 3.0 * mask[:, :, 3]).astype(np.int8)
    t1 = (idx & 1).astype(np.float32)      # [H, I]
    t2 = (idx >> 1).astype(np.float32)
    xp = _pack_imaj(np.ascontiguousarray(x.T), B).astype(fp8)

    nc = _get_nc()
    in_maps = []
    for c in range(NCORES):
        hs = slice(c * H_LOC, (c + 1) * H_LOC)
        par = np.zeros((128, 6 * N_HT), dtype=np.float32)
        for ht in range(N_HT):
            hh = slice(c * H_LOC + ht * 128, c * H_LOC + (ht + 1) * 128)
            par[:, 6 * ht:6 * ht + 4] = tau_n[hh]
            par[:, 6 * ht + 4] = tau_m[hh]
            par[:, 6 * ht + 5] = b[hh]
        # w layout: [128(i in chunk), (chunk, ht, h)]
        wT = np.ascontiguousarray(W[hs].T) * np.float32(SCALE)   # [I, 256]
        wp = wT.reshape(N_CHUNK, 128, N_HT, 128).transpose(1, 0, 2, 3) \
               .reshape(128, N_CHUNK * H_LOC)
        in_maps.append({
            "t1": _pack_hmaj(t1[hs]).astype(fp8),
            "t2": _pack_hmaj(t2[hs]).astype(fp8),
            "w": np.ascontiguousarray(wp).astype(np.float16),
            "x": xp,
            "par": par,
        })

    try:
        res = run_bass_kernel_spmd(
            nc, in_maps, core_ids=list(range(NCORES)), trace=TRACE,
        )
    except Exception:
        if not TRACE:
            raise
        res = run_bass_kernel_spmd(
            nc, in_maps, core_ids=list(range(NCORES)), trace=False,
        )
    LAST_RESULTS = res
    outT = np.concatenate([r["out"].astype(np.float32) for r in res.results], axis=0)
    return np.ascontiguousarray(outT.T)                               # [B, H]
